# revision 1
# baseline (speedup 1.0000x reference)
"""BFP-quantized 3x3 conv (stride 1, pad 1) as im2col matmul on 8 TRN2 cores.

Shapes (hardcoded): inputs [32,128,56,56] f32, weight [256,128,3,3] f32,
bias [256] f32 -> out [32,256,56,56] f32.

Strategy: data-parallel over batch (4 images per core). Host performs
im2col + block-floating-point quantization (block 64 along K=1152,
8-bit signed mantissa). Quantized values are exactly representable in
bf16 (<=8 significand bits), so the device matmul runs in bf16 with
fp32 PSUM accumulation:  outT[256,12544] = qw[256,1152] @ qaT (+ bias
on host), weights stationary, k-innermost, N=512 moving chunks.

The activation matrix is repacked chunk-major on host so each chunk is
a single [128, 9*512] DMA with 9KB contiguous per-partition lines.
Output is stored fp16 (values are O(5), far inside fp16 range; one
rounding at 2^-12 rel) and upcast + bias-added on host.
"""

import numpy as np
import ml_dtypes

import concourse.bacc as bacc
import concourse.mybir as mybir
from concourse.tile import TileContext
from concourse.bass_utils import run_bass_kernel_spmd

N_CORES = 8
N_IMG, C_IN, H, W = 32, 128, 56, 56
C_OUT, KS = 256, 3
K = C_IN * KS * KS            # 1152
PIX = H * W                   # 3136
IMG_PER_CORE = N_IMG // N_CORES
M = IMG_PER_CORE * PIX        # 12544 rows per core
KT = K // 128                 # 9 k-tiles
CB = C_OUT // 128             # 2 c_out blocks
CHUNK = 512
N_CHUNKS = (M + CHUNK - 1) // CHUNK   # 24 full + 1 of 256
AR_COLS = KT * M              # repacked activation columns per partition row

M_BIT, BLOCK = 8, 64

OUT_DTYPE = np.float16  # device-side output dtype (11-bit significand)


def _bfp_quantize_lastaxis(x):
    """Match reference bfp_quantize bit-for-bit in float32 (block 64, m_bit 8)."""
    shape = x.shape
    xb = x.reshape(shape[:-1] + (shape[-1] // BLOCK, BLOCK)).astype(np.float32)
    maxabs = np.max(np.abs(xb), axis=-1, keepdims=True)
    exp = np.floor(np.log2(np.maximum(maxabs, np.float32(1e-38))))
    scale = np.exp2(exp - (M_BIT - 2)).astype(np.float32)
    qmax = np.float32(2.0 ** (M_BIT - 1) - 1)
    q = np.clip(np.round(xb / scale), -qmax - 1.0, qmax).astype(np.float32) * scale
    q = np.where(maxabs == 0.0, np.float32(0.0), q)
    return q.reshape(shape)


_NC_CACHE = {}


def _build_program():
    if "nc" in _NC_CACHE:
        return _NC_CACHE["nc"]
    nc = bacc.Bacc("TRN2")
    bf16 = mybir.dt.bfloat16
    f32 = mybir.dt.float32
    odt = mybir.dt.float16

    aR = nc.dram_tensor("aR", [128, AR_COLS], bf16, kind="ExternalInput")
    wT = nc.dram_tensor("wT", [K, C_OUT], bf16, kind="ExternalInput")
    outT = nc.dram_tensor("outT", [C_OUT, M], odt, kind="ExternalOutput")

    with TileContext(nc) as tc:
        with (
            tc.tile_pool(name="wpool", bufs=1) as wpool,
            tc.tile_pool(name="apool", bufs=4) as apool,
            tc.tile_pool(name="opool", bufs=6) as opool,
            tc.tile_pool(name="pspool", bufs=6, space="PSUM") as pspool,
        ):
            # weights: [1152,256] -> [128 part, (kt, cout)] single DMA
            wtile = wpool.tile([128, KT, C_OUT], bf16)
            nc.sync.dma_start(
                wtile[:, :, :],
                wT[:].rearrange("(kt p) n -> p kt n", p=128),
            )

            for ch in range(N_CHUNKS):
                start = ch * CHUNK
                F = min(CHUNK, M - start)
                atile = apool.tile([128, KT, CHUNK], bf16, tag="a")
                src = aR[:, start * KT : start * KT + KT * F]
                nc.sync.dma_start(
                    atile[:, :, :F],
                    src.rearrange("p (kt m) -> p kt m", kt=KT),
                )
                for cb in range(CB):
                    ps = pspool.tile([128, CHUNK], f32, tag="ps")
                    for kt in range(KT):
                        nc.tensor.matmul(
                            ps[:, :F],
                            wtile[:, kt, cb * 128 : (cb + 1) * 128],
                            atile[:, kt, :F],
                            start=(kt == 0),
                            stop=(kt == KT - 1),
                        )
                    otile = opool.tile([128, CHUNK], odt, tag="o")
                    nc.vector.tensor_copy(otile[:, :F], ps[:, :F])
                    # scalar (ACT) engine queue: keeps output stores off the
                    # SP queue that feeds the activation loads
                    nc.scalar.dma_start(
                        outT[cb * 128 : (cb + 1) * 128, start : start + F],
                        otile[:, :F],
                    )
    if not nc.is_finalized():
        nc.finalize()
    _NC_CACHE["nc"] = nc
    return nc


def _host_prep(inputs, weight, bias):
    """im2col + BFP quantize -> per-core repacked aR [128, KT*M] bf16."""
    x = np.ascontiguousarray(np.asarray(inputs, dtype=np.float32))
    wq = _bfp_quantize_lastaxis(
        np.asarray(weight, dtype=np.float32).reshape(C_OUT, K)
    )
    wT = np.ascontiguousarray(wq.T.astype(ml_dtypes.bfloat16))
    bias_f32 = np.asarray(bias, dtype=np.float32).reshape(C_OUT, 1)

    xp = np.pad(x, ((0, 0), (0, 0), (1, 1), (1, 1)))
    # windows: [N, C, 56, 56, 3, 3]
    win = np.lib.stride_tricks.sliding_window_view(xp, (KS, KS), axis=(2, 3))
    aR_cores = []
    for c in range(N_CORES):
        sl = win[c * IMG_PER_CORE : (c + 1) * IMG_PER_CORE]
        # -> [img, C, kh, kw, 56, 56] -> [img, K, PIX]
        cols = sl.transpose(0, 1, 4, 5, 2, 3).reshape(IMG_PER_CORE, K, PIX)
        # quantize along K for each (img, pix): a is [M, K]
        a = cols.transpose(0, 2, 1).reshape(-1, K)
        qa = _bfp_quantize_lastaxis(a).astype(ml_dtypes.bfloat16)
        # aT3[kt, p, m] = qa[m, kt*128+p]
        aT3 = qa.T.reshape(KT, 128, M)
        # chunk-major repack: aR[p, ch-block] = [kt, m-window] flattened
        parts = []
        for ch in range(N_CHUNKS):
            s = ch * CHUNK
            F = min(CHUNK, M - s)
            parts.append(
                aT3[:, :, s : s + F].transpose(1, 0, 2).reshape(128, KT * F)
            )
        aR_cores.append(np.ascontiguousarray(np.concatenate(parts, axis=1)))
    return aR_cores, wT, bias_f32


def kernel(**inputs):
    aR_cores, wT, bias_f32 = _host_prep(
        inputs["inputs"], inputs["weight"], inputs["bias"]
    )
    nc = _build_program()
    in_maps = [{"aR": aR_cores[c], "wT": wT} for c in range(N_CORES)]
    res = run_bass_kernel_spmd(nc, in_maps, core_ids=list(range(N_CORES)))
    outs = []
    for c in range(N_CORES):
        oT = res.results[c]["outT"].astype(np.float32) + bias_f32  # [256, M]
        outs.append(
            oT.reshape(C_OUT, IMG_PER_CORE, PIX).transpose(1, 0, 2)
        )
    out = np.concatenate(outs, axis=0).reshape(N_IMG, C_OUT, H, W)
    return np.ascontiguousarray(out.astype(np.float32))



# revision 2
# speedup vs baseline: 1.2236x; 1.2236x over previous
"""BFP-quantized 3x3 conv (stride 1, pad 1) as on-the-fly im2col matmul on
8 TRN2 cores, using fp8 DoubleRow matmuls (2 k-tiles per instruction at 0.5
cycles/row = 4x bf16 PE throughput).

Shapes (hardcoded): inputs [32,128,56,56] f32, weight [256,128,3,3] f32,
bias [256] f32 -> out [32,256,56,56] f32.

Strategy: data-parallel over batch (4 images per core). The reference
quantizes both operands to 8-bit-mantissa BFP; we approximate it as:

  out = a8 @ (w_hi + w_lo) + ea8 @ w_hi[compensated positions]

where
  - w_hi + w_lo == qw EXACTLY: the BFP weights' 8-bit mantissas are split
    into two 4-bit nibbles, each exactly representable in fp8 e4m3 after a
    global 2^9 scaling (power-of-two, undone on the host).
  - a8 = e4m3(x) quantized once per input pixel (so im2col can be done
    on the fly from shifted SBUF views -> no 9x HBM blowup),
  - ea8 = e4m3(x - a8) is an fp8 error-compensation plane that cancels the
    activation rounding error on NPOS of the 9 kernel positions.

Each DoubleRow matmul contracts 2 of the (18 + NPOS) k-tiles. PSUM
accumulates in f32; outputs stored f16 (scaled by 2^9), descaled + bias
added on host.
"""

import numpy as np
import ml_dtypes

import concourse.bacc as bacc
import concourse.mybir as mybir
from concourse.tile import TileContext
from concourse.bass_utils import run_bass_kernel_spmd
from bass_rust import AP

FP8 = ml_dtypes.float8_e4m3

N_CORES = 8
N_IMG, C_IN, H, W = 32, 128, 56, 56
C_OUT, KS = 256, 3
IMG_PER_CORE = N_IMG // N_CORES   # 4
PIX = H * W                       # 3136
M = IMG_PER_CORE * PIX            # 12544 output columns per core
K = C_IN * KS * KS                # 1152

HP = H + 2                        # 58 padded
PLANE = HP * HP                   # 3364 elements per partition per plane
ZPAD = 512                        # zero tail per image block (dummy k-tile)
IMG_STRIDE = 2 * PLANE + ZPAD     # 7240
ZOFF = 2 * PLANE                  # zero region offset within image block

ROWS = 8                          # output rows per matmul chunk
MCHUNK = ROWS * W                 # 448 moving rows per DR matmul
NOHB = H // ROWS                  # 7 chunks per image

NPOS = 9                          # kernel positions with act compensation
WSCALE = 512.0                    # global 2^9 weight scaling for fp8 exactness

M_BIT, BLOCK = 8, 64

# k-tile kinds
HI_A, LO_A, HI_E, ZERO = 0, 1, 2, 3


def _pairs(npos):
    """Global (kind, pos) tile order packed into DoubleRow pairs."""
    tiles = []
    for p in range(npos):
        tiles.append((HI_A, p))
        tiles.append((HI_E, p))
    rest = [(HI_A, p) for p in range(npos, KS * KS)]
    rest += [(LO_A, p) for p in range(KS * KS)]
    tiles += rest
    if len(tiles) % 2:
        tiles.append((ZERO, 0))
    return [(tiles[2 * j], tiles[2 * j + 1]) for j in range(len(tiles) // 2)]


PAIRS = _pairs(NPOS)
NPAIR = len(PAIRS)


def _moff(kind, pos, ohb):
    if kind == ZERO:
        return ZOFF
    plane = PLANE if kind == HI_E else 0
    kh, kw = pos // KS, pos % KS
    return plane + (kh + ohb * ROWS) * HP + kw


def _bfp_quantize_lastaxis(x):
    shape = x.shape
    xb = x.reshape(shape[:-1] + (shape[-1] // BLOCK, BLOCK)).astype(np.float32)
    maxabs = np.max(np.abs(xb), axis=-1, keepdims=True)
    exp = np.floor(np.log2(np.maximum(maxabs, np.float32(1e-38))))
    scale = np.exp2(exp - (M_BIT - 2)).astype(np.float32)
    qmax = np.float32(2.0 ** (M_BIT - 1) - 1)
    q = np.clip(np.round(xb / scale), -qmax - 1.0, qmax).astype(np.float32) * scale
    q = np.where(maxabs == 0.0, np.float32(0.0), q)
    return q.reshape(shape), np.repeat(
        scale.reshape(shape[:-1] + (shape[-1] // BLOCK,)), BLOCK, axis=-1
    )


_NC_CACHE = {}


def _build_program():
    if "nc" in _NC_CACHE:
        return _NC_CACHE["nc"]
    nc = bacc.Bacc("TRN2")
    fp8 = mybir.dt.float8e4
    f16 = mybir.dt.float16
    f32 = mybir.dt.float32

    xq = nc.dram_tensor("xq", [128, IMG_PER_CORE * IMG_STRIDE], fp8,
                        kind="ExternalInput")
    wq = nc.dram_tensor("wq", [128, NPAIR, 2, C_OUT], fp8, kind="ExternalInput")
    outT = nc.dram_tensor("outT", [C_OUT, M], f16, kind="ExternalOutput")

    with TileContext(nc) as tc:
        with (
            tc.tile_pool(name="wpool", bufs=1) as wpool,
            tc.tile_pool(name="xpool", bufs=1) as xpool,
            tc.tile_pool(name="opool", bufs=3) as opool,
            tc.tile_pool(name="pspool", bufs=6, space="PSUM") as pspool,
        ):
            wtile = wpool.tile([128, NPAIR, 2, C_OUT], fp8)
            nc.sync.dma_start(wtile[:, :, :, :], wq[:, :, :, :])

            xtiles = []
            for img in range(IMG_PER_CORE):
                xt = xpool.tile([128, IMG_STRIDE], fp8, tag=f"x{img}")
                nc.sync.dma_start(
                    xt[:, :],
                    xq[:, img * IMG_STRIDE : (img + 1) * IMG_STRIDE],
                )
                xtiles.append(xt)

            for img in range(IMG_PER_CORE):
                base = xtiles[img][:, :]
                pstride = base.ap[0][0]
                for ohb in range(NOHB):
                    for cb in range(2):
                        ps = pspool.tile([128, MCHUNK], f32, tag="ps")
                        for j, ((k1, p1), (k2, p2)) in enumerate(PAIRS):
                            o1 = _moff(k1, p1, ohb)
                            o2 = _moff(k2, p2, ohb)
                            mov = AP(
                                base.tensor,
                                o1,
                                [[pstride, 128], [o2 - o1, 2],
                                 [HP, ROWS], [1, W]],
                            )
                            nc.tensor.matmul(
                                ps[:, :],
                                wtile[:, j, :, cb * 128 : (cb + 1) * 128],
                                mov,
                                start=(j == 0),
                                stop=(j == NPAIR - 1),
                                perf_mode=mybir.MatmulPerfMode.DoubleRow,
                            )
                        ot = opool.tile([128, MCHUNK], f16, tag=f"o{cb}")
                        if cb == 0:
                            nc.vector.tensor_copy(ot[:, :], ps[:, :])
                        else:
                            nc.scalar.copy(ot[:, :], ps[:, :])
                        col = img * PIX + ohb * MCHUNK
                        nc.scalar.dma_start(
                            outT[cb * 128 : (cb + 1) * 128,
                                 col : col + MCHUNK],
                            ot[:, :],
                        )
    if not nc.is_finalized():
        nc.finalize()
    _NC_CACHE["nc"] = nc
    return nc


def _host_prep(inputs, weight, bias):
    x = np.asarray(inputs, dtype=np.float32)
    # padded activations + fp8 planes (quantized once per input pixel)
    xp = np.zeros((N_IMG, C_IN, HP, HP), dtype=np.float32)
    xp[:, :, 1:-1, 1:-1] = x
    a8 = xp.astype(FP8)
    ea8 = (xp - a8.astype(np.float32)).astype(FP8)

    xq_cores = []
    for c in range(N_CORES):
        arr = np.zeros((128, IMG_PER_CORE, IMG_STRIDE), dtype=FP8)
        sl = slice(c * IMG_PER_CORE, (c + 1) * IMG_PER_CORE)
        # [img, C, HP, HP] -> [C, img, PLANE]
        arr[:, :, :PLANE] = a8[sl].reshape(
            IMG_PER_CORE, 128, PLANE).transpose(1, 0, 2)
        arr[:, :, PLANE : 2 * PLANE] = ea8[sl].reshape(
            IMG_PER_CORE, 128, PLANE).transpose(1, 0, 2)
        xq_cores.append(
            np.ascontiguousarray(arr.reshape(128, IMG_PER_CORE * IMG_STRIDE))
        )

    # weights: reference BFP quantization (exact), nibble split, 2^9 scale
    qw, ws = _bfp_quantize_lastaxis(
        np.asarray(weight, dtype=np.float32).reshape(C_OUT, K)
    )
    i = np.round(qw / ws)
    i_hi = np.round(i / 16.0)
    w_hi = (16.0 * i_hi * ws * WSCALE).astype(np.float32)
    w_lo = ((i - 16.0 * i_hi) * ws * WSCALE).astype(np.float32)
    # [C_OUT, K] -> [128 (c_in), 9 (pos), C_OUT]
    w_hi_t = w_hi.reshape(C_OUT, C_IN, KS * KS).transpose(1, 2, 0)
    w_lo_t = w_lo.reshape(C_OUT, C_IN, KS * KS).transpose(1, 2, 0)
    wq = np.zeros((128, NPAIR, 2, C_OUT), dtype=np.float32)
    for j, pair in enumerate(PAIRS):
        for slot, (kind, pos) in enumerate(pair):
            if kind == ZERO:
                continue
            src = w_lo_t if kind == LO_A else w_hi_t
            wq[:, j, slot, :] = src[:, pos, :]
    wq8 = wq.astype(FP8)
    assert np.array_equal(wq8.astype(np.float32), wq), "fp8 weight split inexact"

    bias_f32 = np.asarray(bias, dtype=np.float32).reshape(C_OUT, 1)
    return xq_cores, wq8, bias_f32


def kernel(**inputs):
    xq_cores, wq8, bias_f32 = _host_prep(
        inputs["inputs"], inputs["weight"], inputs["bias"]
    )
    nc = _build_program()
    in_maps = [{"xq": xq_cores[c], "wq": wq8} for c in range(N_CORES)]
    res = run_bass_kernel_spmd(nc, in_maps, core_ids=list(range(N_CORES)))
    outs = []
    for c in range(N_CORES):
        oT = res.results[c]["outT"].astype(np.float32) / WSCALE + bias_f32
        outs.append(oT.reshape(C_OUT, IMG_PER_CORE, PIX).transpose(1, 0, 2))
    out = np.concatenate(outs, axis=0).reshape(N_IMG, C_OUT, H, W)
    return np.ascontiguousarray(out.astype(np.float32))


# revision 42
# speedup vs baseline: 1.2792x; 1.0455x over previous
"""BFP-quantized 3x3 conv (stride 1, pad 1) as on-the-fly im2col matmul on
8 TRN2 cores, using fp8 DoubleRow matmuls (2 k-tiles per instruction at 0.5
cycles/row = 4x bf16 PE throughput).

Shapes (hardcoded): inputs [32,128,56,56] f32, weight [256,128,3,3] f32,
bias [256] f32 -> out [32,256,56,56] f32.

Strategy: data-parallel over batch (4 images per core). The reference
quantizes both operands to 8-bit-mantissa BFP; we approximate it as:

  out = a8 @ (w_hi + w_lo) + ea8 @ w_hi[compensated positions]

where
  - w_hi + w_lo == qw EXACTLY: the BFP weights' 8-bit mantissas are split
    into two 4-bit nibbles, each exactly representable in fp8 e4m3 after a
    global 2^9 scaling (power-of-two, undone on the host).
  - a8 = e4m3(x) quantized once per input pixel (so im2col can be done
    on the fly from shifted SBUF views -> no 9x HBM blowup),
  - ea8 = e4m3(x - a8) is an fp8 error-compensation plane that cancels the
    activation rounding error on NPOS of the 9 kernel positions.

Each DoubleRow matmul contracts 2 of the (18 + NPOS) k-tiles. PSUM
accumulates in f32; outputs stored f16 (scaled by 2^9), descaled + bias
added on host.
"""

import numpy as np
import ml_dtypes

import concourse.bacc as bacc
import concourse.mybir as mybir
from concourse.tile import TileContext
from concourse.bass_utils import run_bass_kernel_spmd
from bass_rust import AP

FP8 = ml_dtypes.float8_e4m3

N_CORES = 8
N_IMG, C_IN, H, W = 32, 128, 56, 56
C_OUT, KS = 256, 3
IMG_PER_CORE = N_IMG // N_CORES   # 4
PIX = H * W                       # 3136
M = IMG_PER_CORE * PIX            # 12544 output columns per core
K = C_IN * KS * KS                # 1152

HP = H + 2                        # 58 padded
PLANE = HP * HP                   # 3364 elements per partition per plane
ZPAD = 512                        # zero tail per image block (dummy k-tile)
IMG_STRIDE = 2 * PLANE + ZPAD     # 7240
ZOFF = 2 * PLANE                  # zero region offset within image block

ROWS = 8                          # output rows per matmul chunk
MCHUNK = ROWS * W                 # 448 moving rows per DR matmul
NOHB = H // ROWS                  # 7 chunks per image

WSCALE = 512.0                    # global 2^9 weight scaling for fp8 exactness

M_BIT, BLOCK = 8, 64

# k-tile kinds
HI_A, LO_A, HI_E, ZERO = 0, 1, 2, 3

# 27 k-tiles + 1 zero tile in 14 DoubleRow pairs (full activation
# compensation -> measured rel err 0.0095 vs gate 0.02). NOTE: odd-length
# accumulation chains (13 matmuls) crash the device; keep NPAIR even.
# Every within-pair stride is positive; a8-plane pairs come first.
PAIRS = (
    [((HI_A, p), (LO_A, p + 1)) for p in range(KS * KS - 1)]
    + [((LO_A, 0), (HI_A, 8))]
    + [((HI_E, 2 * q), (HI_E, 2 * q + 1)) for q in range(4)]
    + [((HI_E, 8), (ZERO, 0))]
)
NPAIR = len(PAIRS)


def _moff(kind, pos, ohb, eoff, zoff):
    if kind == ZERO:
        return zoff
    kh, kw = pos // KS, pos % KS
    return (eoff if kind == HI_E else 0) + (kh + ohb * ROWS) * HP + kw


def _bfp_quantize_lastaxis(x):
    shape = x.shape
    xb = x.reshape(shape[:-1] + (shape[-1] // BLOCK, BLOCK)).astype(np.float32)
    maxabs = np.max(np.abs(xb), axis=-1, keepdims=True)
    exp = np.floor(np.log2(np.maximum(maxabs, np.float32(1e-38))))
    scale = np.exp2(exp - (M_BIT - 2)).astype(np.float32)
    qmax = np.float32(2.0 ** (M_BIT - 1) - 1)
    q = np.clip(np.round(xb / scale), -qmax - 1.0, qmax).astype(np.float32) * scale
    q = np.where(maxabs == 0.0, np.float32(0.0), q)
    return q.reshape(shape), np.repeat(
        scale.reshape(shape[:-1] + (shape[-1] // BLOCK,)), BLOCK, axis=-1
    )


_NC_CACHE = {}


def _build_program():
    if "nc" in _NC_CACHE:
        return _NC_CACHE["nc"]
    nc = bacc.Bacc("TRN2")
    fp8 = mybir.dt.float8e4
    f16 = mybir.dt.float16
    f32 = mybir.dt.float32

    WQ_SPLIT = bool(_NC_CACHE.get("wq_split", True))
    xq = nc.dram_tensor("xq", [128, IMG_PER_CORE * IMG_STRIDE], fp8,
                        kind="ExternalInput")
    if WQ_SPLIT:
        wq0 = nc.dram_tensor("wq0", [128, NPAIR, 2, 128], fp8,
                             kind="ExternalInput")
        wq1 = nc.dram_tensor("wq1", [128, NPAIR, 2, 128], fp8,
                             kind="ExternalInput")
    else:
        wq0 = nc.dram_tensor("wq0", [128, NPAIR, 2, C_OUT], fp8,
                             kind="ExternalInput")
    outT = nc.dram_tensor("outT", [C_OUT, M], f16, kind="ExternalOutput")

    N_WARM = int(_NC_CACHE.get("n_warm", 16))
    USE_BAND = bool(_NC_CACHE.get("use_band", True))
    N_IMG_BUILD = int(_NC_CACHE.get("n_img", IMG_PER_CORE))
    HEAD = (ROWS + 2) * HP

    with TileContext(nc) as tc:
        with (
            tc.tile_pool(name="wpool", bufs=1) as wpool,
            tc.tile_pool(name="xpool", bufs=1) as xpool,
            tc.tile_pool(name="opool", bufs=3) as opool,
            tc.tile_pool(name="pspool", bufs=6, space="PSUM") as pspool,
        ):
            # PE warmup: dummy DoubleRow matmuls on a zeroed scratch tile keep
            # the tensor engine busy through its p-state ramp while the first
            # input/weight DMAs are in flight.
            if N_WARM:
                dummy = wpool.tile([128, 192], fp8, tag="dummy")
                nc.vector.memset(dummy[:, :], 0.0)
                dps = pspool.tile([128, MCHUNK], f32, tag="ps")
                dmov = AP(
                    dummy[:, :].tensor, 0,
                    [[dummy[:, :].ap[0][0], 128], [1, 2], [1, ROWS], [1, W]],
                )
                dw = AP(
                    dummy[:, :].tensor, 0,
                    [[dummy[:, :].ap[0][0], 128], [64, 2], [1, 128]],
                )
                for _ in range(N_WARM):
                    nc.tensor.matmul(
                        dps[:, :], dw, dmov, start=True, stop=True,
                        perf_mode=mybir.MatmulPerfMode.DoubleRow,
                    )

            # image-0 head band (rows 0..9 of both planes, one tile), then
            # the two weight halves, then full combined [a8|ea8] image tiles.
            xb0 = None
            if USE_BAND:
                # [a8 head | ea8 head | zeros], all from image 0's dram block
                xq_base = xq[:, :]
                dstride = xq_base.ap[0][0]
                xb0 = xpool.tile([128, 2 * HEAD + ZPAD], fp8, tag="xb0")
                nc.sync.dma_start(
                    xb0[:, : 2 * HEAD].rearrange("p (t q) -> p t q", t=2),
                    AP(xq_base.tensor, 0,
                       [[dstride, 128], [PLANE, 2], [1, HEAD]]),
                )
                nc.sync.dma_start(xb0[:, 2 * HEAD :], xq[:, ZOFF:IMG_STRIDE])
            if WQ_SPLIT:
                wt0 = wpool.tile([128, NPAIR, 2, 128], fp8, tag="w0")
                nc.sync.dma_start(wt0[:, :, :, :], wq0[:, :, :, :])
                wt1 = wpool.tile([128, NPAIR, 2, 128], fp8, tag="w1")
                nc.sync.dma_start(wt1[:, :, :, :], wq1[:, :, :, :])

                def wslice(cb, j):
                    return (wt0 if cb == 0 else wt1)[:, j, :, :]
            else:
                wtc = wpool.tile([128, NPAIR, 2, C_OUT], fp8, tag="w0")
                nc.sync.dma_start(wtc[:, :, :, :], wq0[:, :, :, :])

                def wslice(cb, j):
                    return wtc[:, j, :, cb * 128 : (cb + 1) * 128]
            xc = []
            for img in range(N_IMG_BUILD):
                xci = xpool.tile([128, IMG_STRIDE], fp8, tag=f"xc{img}")
                nc.sync.dma_start(
                    xci[:, :],
                    xq[:, img * IMG_STRIDE : (img + 1) * IMG_STRIDE],
                )
                xc.append(xci)

            for img in range(N_IMG_BUILD):
                for ohb in range(NOHB):
                    if USE_BAND and img == 0 and ohb == 0:
                        base, eoff, zoff = xb0[:, :], HEAD, 2 * HEAD
                    else:
                        base, eoff, zoff = xc[img][:, :], PLANE, ZOFF
                    for cb in range(2):
                        ps = pspool.tile([128, MCHUNK], f32, tag="ps")
                        for j, ((k1, p1), (k2, p2)) in enumerate(PAIRS):
                            o1 = _moff(k1, p1, ohb, eoff, zoff)
                            o2 = _moff(k2, p2, ohb, eoff, zoff)
                            mov = AP(
                                base.tensor,
                                o1,
                                [[base.ap[0][0], 128], [o2 - o1, 2],
                                 [HP, ROWS], [1, W]],
                            )
                            nc.tensor.matmul(
                                ps[:, :],
                                wslice(cb, j),
                                mov,
                                start=(j == 0),
                                stop=(j == NPAIR - 1),
                                perf_mode=mybir.MatmulPerfMode.DoubleRow,
                            )
                        ot = opool.tile([128, MCHUNK], f16, tag=f"o{cb}")
                        nc.vector.tensor_copy(ot[:, :], ps[:, :])
                        col = img * PIX + ohb * MCHUNK
                        (nc.sync if cb == 0 else nc.scalar).dma_start(
                            outT[cb * 128 : (cb + 1) * 128,
                                 col : col + MCHUNK],
                            ot[:, :],
                        )
    if not nc.is_finalized():
        nc.finalize()
    _NC_CACHE["nc"] = nc
    return nc


def _host_prep(inputs, weight, bias):
    x = np.asarray(inputs, dtype=np.float32)
    # padded activations + fp8 planes (quantized once per input pixel)
    xp = np.zeros((N_IMG, C_IN, HP, HP), dtype=np.float32)
    xp[:, :, 1:-1, 1:-1] = x
    a8 = xp.astype(FP8)
    ea8 = (xp - a8.astype(np.float32)).astype(FP8)

    xq_cores = []
    for c in range(N_CORES):
        arr = np.zeros((128, IMG_PER_CORE, IMG_STRIDE), dtype=FP8)
        sl = slice(c * IMG_PER_CORE, (c + 1) * IMG_PER_CORE)
        # [img, C, HP, HP] -> [C, img, PLANE]
        arr[:, :, :PLANE] = a8[sl].reshape(
            IMG_PER_CORE, 128, PLANE).transpose(1, 0, 2)
        arr[:, :, PLANE : 2 * PLANE] = ea8[sl].reshape(
            IMG_PER_CORE, 128, PLANE).transpose(1, 0, 2)
        xq_cores.append(
            np.ascontiguousarray(arr.reshape(128, IMG_PER_CORE * IMG_STRIDE))
        )

    # weights: reference BFP quantization (exact), nibble split, 2^9 scale
    qw, ws = _bfp_quantize_lastaxis(
        np.asarray(weight, dtype=np.float32).reshape(C_OUT, K)
    )
    i = np.round(qw / ws)
    i_hi = np.round(i / 16.0)
    w_hi = (16.0 * i_hi * ws * WSCALE).astype(np.float32)
    w_lo = ((i - 16.0 * i_hi) * ws * WSCALE).astype(np.float32)
    # [C_OUT, K] -> [128 (c_in), 9 (pos), C_OUT]
    w_hi_t = w_hi.reshape(C_OUT, C_IN, KS * KS).transpose(1, 2, 0)
    w_lo_t = w_lo.reshape(C_OUT, C_IN, KS * KS).transpose(1, 2, 0)
    wq = np.zeros((128, NPAIR, 2, C_OUT), dtype=np.float32)
    for j, pair in enumerate(PAIRS):
        for slot, (kind, pos) in enumerate(pair):
            if kind == ZERO:
                continue
            src = w_lo_t if kind == LO_A else w_hi_t
            wq[:, j, slot, :] = src[:, pos, :]
    wq8 = wq.astype(FP8)
    assert np.array_equal(wq8.astype(np.float32), wq), "fp8 weight split inexact"
    wq8_0 = np.ascontiguousarray(wq8[:, :, :, :128])
    wq8_1 = np.ascontiguousarray(wq8[:, :, :, 128:])

    bias_f32 = np.asarray(bias, dtype=np.float32).reshape(C_OUT, 1)
    return xq_cores, wq8_0, wq8_1, bias_f32


def kernel(**inputs):
    xq_cores, wq8_0, wq8_1, bias_f32 = _host_prep(
        inputs["inputs"], inputs["weight"], inputs["bias"]
    )
    nc = _build_program()
    if _NC_CACHE.get("wq_split", True):
        in_maps = [
            {"xq": xq_cores[c], "wq0": wq8_0, "wq1": wq8_1}
            for c in range(N_CORES)
        ]
    else:
        wq8_c = np.ascontiguousarray(np.concatenate([wq8_0, wq8_1], axis=3))
        in_maps = [{"xq": xq_cores[c], "wq0": wq8_c} for c in range(N_CORES)]
    res = run_bass_kernel_spmd(nc, in_maps, core_ids=list(range(N_CORES)))
    outs = []
    for c in range(N_CORES):
        oT = res.results[c]["outT"].astype(np.float32) / WSCALE + bias_f32
        outs.append(oT.reshape(C_OUT, IMG_PER_CORE, PIX).transpose(1, 0, 2))
    out = np.concatenate(outs, axis=0).reshape(N_IMG, C_OUT, H, W)
    return np.ascontiguousarray(out.astype(np.float32))
